# revision 3
# baseline (speedup 1.0000x reference)
"""Trainium2 Bass kernel for per-token multi-head cross attention.

Math (per token t):
    q = x Wq, k = c Wk, v = c Wv                  (512 -> 8 heads x 64)
    S[h,g] = sum_d q[h,d] k[g,d]                  (8x8 per token)
    P = softmax(S, axis=g)
    o[h,:] = sum_g P[h,g] v[g,:]
    out = o Wo + bo

Sharding: data-parallel over the flattened token axis (B*N = 32768) across
8 cores, 4096 tokens each.  Weights replicated.  No collectives.

Per-core layout: 32 tiles of 128 tokens.
  PE:  transpose x/c tiles, Q/K/V projections (f32), transpose of attn
       output, final projection with bias folded in as a K=1 matmul.
  DVE: per-token scores via broadcast muls + in-place binary-tree
       reductions (fp16, 2x mode), softmax smalls, PV muls + tree.
  ACT: PSUM evacuations (with f32->fp16 casts) and exp.

V is projected with host-permuted weight columns (d*8+g instead of
g*64+d) so the PV multiplies have a unit-stride innermost dim on both
inputs (required for the DVE 2x perf mode).
"""

import sys

sys.path.insert(0, "/opt/trn_rl_repo")

import numpy as np
import ml_dtypes

import concourse.bass as bass
from concourse import bacc
import concourse.tile as tile
from concourse import mybir
from concourse.bass import ts
from concourse.bass_utils import run_bass_kernel_spmd
from concourse.masks import make_identity

F32 = mybir.dt.float32
F16 = mybir.dt.float16
BF16 = mybir.dt.bfloat16

N_CORES = 8
TOK_PER_CORE = 4096
D = 512
H = 8
DH = 64
P = 128  # tokens per tile
N_TILES = TOK_PER_CORE // P

TRACE = False
TRACE_TMPDIR = None
LAST_EXEC_NS = None

Exp = mybir.ActivationFunctionType.Exp
Copy = mybir.ActivationFunctionType.Copy
X = mybir.AxisListType.X
ADD = mybir.AluOpType.add
MAX = mybir.AluOpType.max


def build_bass(repeat=1):
    nc = bacc.Bacc("TRN2")

    x_d = nc.dram_tensor("x", [TOK_PER_CORE, D], F32, kind="ExternalInput")
    c_d = nc.dram_tensor("cx", [TOK_PER_CORE, D], F32, kind="ExternalInput")
    wq_d = nc.dram_tensor("wq", [D, D], BF16, kind="ExternalInput")
    wk_d = nc.dram_tensor("wk", [D, D], BF16, kind="ExternalInput")
    wv_d = nc.dram_tensor("wv", [D, D], BF16, kind="ExternalInput")
    wo_d = nc.dram_tensor("wo", [D, D], BF16, kind="ExternalInput")
    bo_d = nc.dram_tensor("bo", [1, D], BF16, kind="ExternalInput")
    out_d = nc.dram_tensor("out", [TOK_PER_CORE, D], F32, kind="ExternalOutput")

    TPP = 16  # token tiles per pass (2 passes of 2048 tokens)

    with tile.TileContext(nc) as tc:
        with (
            tc.tile_pool(name="singles", bufs=1) as singles,
            tc.tile_pool(name="io", bufs=1) as io,
            tc.tile_pool(name="work", bufs=2) as work,
            tc.tile_pool(name="psum", bufs=1, space="PSUM") as psum,
        ):
            # ---- constants / weights (loaded once) ----
            id32 = singles.tile([P, P], F32, tag="id32")
            make_identity(nc, id32)
            id16 = singles.tile([P, P], BF16, tag="id16")
            make_identity(nc, id16)
            ones16 = singles.tile([1, P], BF16, tag="ones16")
            nc.vector.memset(ones16, 1.0)

            wq_s = singles.tile([P, 4, D], BF16, tag="wq_s")
            nc.sync.dma_start(out=wq_s, in_=wq_d[:].rearrange("(k p) j -> p k j", p=P))
            wk_s = singles.tile([P, 4, D], BF16, tag="wk_s")
            nc.sync.dma_start(out=wk_s, in_=wk_d[:].rearrange("(k p) j -> p k j", p=P))
            wv_s = singles.tile([P, 4, D], BF16, tag="wv_s")
            nc.sync.dma_start(out=wv_s, in_=wv_d[:].rearrange("(k p) j -> p k j", p=P))
            wo_s = singles.tile([P, 4, D], BF16, tag="wo_s")
            nc.sync.dma_start(out=wo_s, in_=wo_d[:].rearrange("(k p) j -> p k j", p=P))
            bo_s = singles.tile([1, D], BF16, tag="bo_s")
            nc.sync.dma_start(out=bo_s, in_=bo_d[:])

            rep_ctx = tc.For_i(0, repeat, 1) if repeat > 1 else None
            if rep_ctx is not None:
                rep_ctx.__enter__()
            for ps in range(N_TILES // TPP):
                # persistent per-pass IO buffers (fixed addresses; their
                # only readers/writers keep DMA wait counts at <=2)
                x_buf = io.tile([P, TPP, D], F32, tag="x_buf")
                c_buf = io.tile([P, TPP, D], F32, tag="c_buf")
                out_buf = io.tile([P, TPP, D], F32, tag="out_buf")

                for t in range(TPP):
                    i = ps * TPP + t
                    tok = ts(i, P)

                    # ---- load (into persistent slices) ----
                    nc.sync.dma_start(out=x_buf[:, t, :], in_=x_d[tok, :])
                    nc.sync.dma_start(out=c_buf[:, t, :], in_=c_d[tok, :])

                    # ---- transpose x, c (PE) ----
                    xt_ps = psum.tile([P, D], F32, tag="xt_ps")
                    ct_ps = psum.tile([P, D], F32, tag="ct_ps")
                    for k in range(4):
                        nc.tensor.transpose(xt_ps[:, ts(k, P)], x_buf[:, t, ts(k, P)], id32)
                        nc.tensor.transpose(ct_ps[:, ts(k, P)], c_buf[:, t, ts(k, P)], id32)
                    xt = work.tile([P, D], BF16, tag="xt")
                    nc.scalar.activation(out=xt, in_=xt_ps, func=Copy)
                    ct = work.tile([P, D], BF16, tag="ct")
                    nc.scalar.activation(out=ct, in_=ct_ps, func=Copy)

                    # ---- projections (PE, f32) ----
                    q_ps = psum.tile([P, D], F32, tag="q_ps")
                    k_ps = psum.tile([P, D], F32, tag="k_ps")
                    v_ps = psum.tile([P, D], F32, tag="v_ps")
                    for k in range(4):
                        nc.tensor.matmul(q_ps, xt[:, ts(k, P)], wq_s[:, k, :],
                                         start=(k == 0), stop=(k == 3))
                    for k in range(4):
                        nc.tensor.matmul(k_ps, ct[:, ts(k, P)], wk_s[:, k, :],
                                         start=(k == 0), stop=(k == 3))
                    for k in range(4):
                        nc.tensor.matmul(v_ps, ct[:, ts(k, P)], wv_s[:, k, :],
                                         start=(k == 0), stop=(k == 3))

                    q16 = work.tile([P, D], F16, tag="q16")  # (t, (h,d))
                    nc.scalar.activation(out=q16, in_=q_ps, func=Copy)
                    k16 = work.tile([P, D], F16, tag="k16")  # (t, (g,d))
                    nc.scalar.activation(out=k16, in_=k_ps, func=Copy)
                    v16 = work.tile([P, D], BF16, tag="v16")  # (t, (d,g)) [wv perm]
                    nc.scalar.activation(out=v16, in_=v_ps, func=Copy)

                    qv = q16[:].rearrange("p (h d) -> p h d", h=H)
                    kv = k16[:].rearrange("p (g d) -> p g d", g=H)
                    vv = v16[:].rearrange("p (d g) -> p d g", d=DH)

                    # ---- scores: S[t,h,g] = sum_d q k  (DVE fp16) ----
                    # single 4D-AP mul: q broadcast over g, k broadcast over h
                    prod = work.tile([P, H, H, DH], F16, tag="prod")  # (t,h,g,d)
                    nc.vector.tensor_mul(
                        prod,
                        qv.unsqueeze(2).to_broadcast([P, H, H, DH]),
                        kv.unsqueeze(1).to_broadcast([P, H, H, DH]),
                    )
                    w = DH // 2
                    while w >= 2:
                        nc.vector.tensor_add(
                            prod[:, :, :, 0:w], prod[:, :, :, 0:w],
                            prod[:, :, :, w : 2 * w]
                        )
                        w //= 2
                    s32 = work.tile([P, H, H], F32, tag="s32")
                    nc.vector.tensor_add(s32.unsqueeze(3), prod[:, :, :, 0:1],
                                         prod[:, :, :, 1:2])

                    # ---- softmax over g ----
                    mx = work.tile([P, H], F32, tag="mx")
                    nc.vector.tensor_reduce(mx, s32, axis=X, op=MAX)
                    nc.vector.tensor_sub(s32, s32,
                                         mx.unsqueeze(2).to_broadcast([P, H, H]))
                    p16 = work.tile([P, H, H], BF16, tag="p16")
                    nc.scalar.activation(out=p16, in_=s32, func=Exp)
                    dn = work.tile([P, H], F32, tag="dn")
                    nc.vector.tensor_reduce(dn, p16, axis=X, op=ADD)
                    rc = work.tile([P, H], F32, tag="rc")
                    nc.vector.reciprocal(rc, dn)
                    rc16 = work.tile([P, H], BF16, tag="rc16")
                    nc.scalar.activation(out=rc16, in_=rc, func=Copy)
                    nc.vector.tensor_mul(
                        p16, p16, rc16.unsqueeze(2).to_broadcast([P, H, H])
                    )

                    # ---- PV: o[t,h,d] = sum_g P V  (DVE fp16) ----
                    prod2 = work.tile([P, H, DH, H], BF16, tag="prod2")  # (t,h,d,g)
                    nc.vector.tensor_mul(
                        prod2,
                        p16.unsqueeze(2).to_broadcast([P, H, DH, H]),
                        vv.unsqueeze(1).to_broadcast([P, H, DH, H]),
                    )
                    w = H // 2
                    while w >= 2:
                        nc.vector.tensor_add(
                            prod2[:, :, :, 0:w], prod2[:, :, :, 0:w],
                            prod2[:, :, :, w : 2 * w],
                        )
                        w //= 2
                    o2 = work.tile([P, D], BF16, tag="o2")  # (t, (h,d))
                    o2v = o2[:].rearrange("p (h d) -> p h d", h=H).unsqueeze(3)
                    nc.vector.tensor_add(o2v, prod2[:, :, :, 0:1], prod2[:, :, :, 1:2])

                    # ---- output projection ----
                    ot_ps = psum.tile([P, D], BF16, tag="ot_ps")
                    for k in range(4):
                        nc.tensor.transpose(ot_ps[:, ts(k, P)], o2[:, ts(k, P)], id16)
                    ot16 = work.tile([P, D], BF16, tag="ot16")
                    nc.scalar.activation(out=ot16, in_=ot_ps, func=Copy)

                    o_ps = psum.tile([P, D], F32, tag="o_ps")
                    for k in range(4):
                        nc.tensor.matmul(o_ps, ot16[:, ts(k, P)], wo_s[:, k, :],
                                         start=(k == 0), stop=False)
                    nc.tensor.matmul(o_ps, ones16, bo_s, start=False, stop=True)

                    nc.scalar.activation(out=out_buf[:, t, :], in_=o_ps, func=Copy)
                    nc.sync.dma_start(out=out_d[tok, :], in_=out_buf[:, t, :])

            if rep_ctx is not None:
                rep_ctx.__exit__(None, None, None)

    nc.finalize()
    return nc


_NC = None


def prep_in_maps(x, context, Wq, Wk, Wv, Wo, bo):
    x = np.asarray(x, dtype=np.float32).reshape(-1, D)
    c = np.asarray(context, dtype=np.float32).reshape(-1, D)
    wq = np.ascontiguousarray(np.asarray(Wq, dtype=np.float32).astype(ml_dtypes.bfloat16))
    wk = np.ascontiguousarray(np.asarray(Wk, dtype=np.float32).astype(ml_dtypes.bfloat16))
    # permute V columns: g*64+d -> d*8+g
    wv = np.asarray(Wv, dtype=np.float32).reshape(D, H, DH)
    wv = np.ascontiguousarray(wv.transpose(0, 2, 1).reshape(D, D).astype(ml_dtypes.bfloat16))
    wo = np.ascontiguousarray(np.asarray(Wo, dtype=np.float32).astype(ml_dtypes.bfloat16))
    bo_ = np.ascontiguousarray(np.asarray(bo, dtype=np.float32).astype(ml_dtypes.bfloat16).reshape(1, D))
    n_tok = x.shape[0]
    per = n_tok // N_CORES
    assert per == TOK_PER_CORE, (n_tok, TOK_PER_CORE)
    in_maps = []
    for i in range(N_CORES):
        sl = slice(i * per, (i + 1) * per)
        in_maps.append(
            {
                "x": np.ascontiguousarray(x[sl]),
                "cx": np.ascontiguousarray(c[sl]),
                "wq": wq,
                "wk": wk,
                "wv": wv,
                "wo": wo,
                "bo": bo_,
            }
        )
    return in_maps


def kernel(x, context, Wq, Wk, Wv, Wo, bo):
    global _NC, LAST_EXEC_NS
    in_maps = prep_in_maps(x, context, Wq, Wk, Wv, Wo, bo)

    if _NC is None:
        _NC = build_bass()

    res = run_bass_kernel_spmd(
        _NC, in_maps, list(range(N_CORES)), trace=TRACE, tmpdir=TRACE_TMPDIR
    )
    LAST_EXEC_NS = res.exec_time_ns
    out = np.concatenate([res.results[i]["out"] for i in range(N_CORES)], axis=0)
    return out.reshape(8, 4096, D).astype(np.float32)

